# revision 17
# baseline (speedup 1.0000x reference)
"""ContextGateAttention TRN2 kernel.

B=4, L=512, D=128. 8 NeuronCores, data-parallel: core c owns batch c//2,
query rows (c%2)*256 .. +256. No collectives.

Per-core pipeline (all in [partition, free] layouts):
  KT/QT       : PE transposes of K,Q tiles              -> [d, j], [d, i]
  hqT         : W1a^T-matmul                            -> [d2, i]  (256)
  hkbT        : W1b-matmul + b1 bias                    -> [d2, j]  (512)
  scores      : QT^T @ KT, scaled by 0.5/sqrt(D)        -> [i, j]
  conflict    : per query i: relu(hkbT + hqT[:,i]) on DVE/ACT (fused
                tensor_scalar add+max / activation Relu+bias), then a
                M=1 PE matmul against w2 reduces over d -> PSUM row i
  gate        : sigmoid(c + b2) = 0.5*(1 + tanh(c/2 + b2/2)); with the
                0.5 folded into the score scale: gated = s*t + s
  softmax     : reduce_max -> exp(bias=-max, accum_out=sum) -> mul 1/sum
  out         : PE transpose of attn -> attn^T @ V (4-step PSUM accum)
"""

import math
from contextlib import ExitStack

import numpy as np

import concourse.bass as bass
import concourse.bacc as bacc
import concourse.mybir as mybir
import concourse.tile as tile
from concourse.bass_utils import run_bass_kernel_spmd
from concourse.masks import make_identity

B, L, D = 4, 512, 128
NCORES = 8
IPC = L * B // NCORES  # 256 query rows per core
NCHUNK = IPC // 128  # 2 partition chunks of query rows
FT = mybir.dt.float32
BF = mybir.dt.float16
AF = mybir.ActivationFunctionType
ALU = mybir.AluOpType

_module_cache: dict = {}


def _build_module(masked: bool) -> bass.Bass:
    nc = bacc.Bacc()
    q = nc.declare_dram_parameter("q", [IPC, D], FT, isOutput=False)
    k = nc.declare_dram_parameter("k", [L, D], FT, isOutput=False)
    v = nc.declare_dram_parameter("v", [L, D], FT, isOutput=False)
    w1a = nc.declare_dram_parameter("w1a", [D, D], FT, isOutput=False)
    w1b = nc.declare_dram_parameter("w1b", [D, D], FT, isOutput=False)
    w2 = nc.declare_dram_parameter("w2", [D, 1], FT, isOutput=False)
    b1 = nc.declare_dram_parameter("b1", [D, 1], FT, isOutput=False)
    b2h = nc.declare_dram_parameter("b2h", [D, 1], FT, isOutput=False)
    am = (
        nc.declare_dram_parameter("am", [IPC, L], FT, isOutput=False)
        if masked
        else None
    )
    attn = nc.declare_dram_parameter("attn", [IPC, L], FT, isOutput=True)
    out = nc.declare_dram_parameter("out", [IPC, D], FT, isOutput=True)

    with tile.TileContext(nc) as tc:
        with ExitStack() as ctx:
            _body(ctx, tc, q, k, v, w1a, w1b, w2, b1, b2h, am, attn, out)
    nc.compile()
    return nc


def _body(ctx, tc, q, k, v, w1a, w1b, w2, b1, b2h, am, attn, out):
    nc = tc.nc
    JCH = L // 128  # 4 key chunks
    dmae = [nc.sync, nc.gpsimd, nc.scalar]

    const = ctx.enter_context(tc.tile_pool(name="const", bufs=1))
    sb_in = ctx.enter_context(tc.tile_pool(name="sb_in", bufs=1))
    sb_mid = ctx.enter_context(tc.tile_pool(name="sb_mid", bufs=1))
    relu_p = ctx.enter_context(tc.tile_pool(name="relu", bufs=10))
    tmp_p = ctx.enter_context(tc.tile_pool(name="tmp", bufs=4))
    small_p = ctx.enter_context(tc.tile_pool(name="small", bufs=8))
    attn_p = ctx.enter_context(tc.tile_pool(name="attn", bufs=2))
    att_t = ctx.enter_context(tc.tile_pool(name="att_t", bufs=8))
    ps_big = ctx.enter_context(tc.tile_pool(name="ps_big", bufs=4, space="PSUM"))
    ps_sm = ctx.enter_context(tc.tile_pool(name="ps_sm", bufs=2, space="PSUM"))

    # ---- input DMAs first, spread across engine queues ----
    k_sb = sb_in.tile([128, JCH, D], FT)
    kr = k[:, :].rearrange("(c p) d -> p c d", c=JCH)
    for jc in range(JCH):
        dmae[jc % 3].dma_start(out=k_sb[:, jc, :], in_=kr[:, jc, :])
    q_sb = sb_in.tile([128, NCHUNK, D], FT)
    qr = q[:, :].rearrange("(c p) d -> p c d", c=NCHUNK)
    for icn in range(NCHUNK):
        dmae[icn % 3].dma_start(out=q_sb[:, icn, :], in_=qr[:, icn, :])
    w1a_sb = const.tile([128, D], FT)
    nc.sync.dma_start(out=w1a_sb, in_=w1a[:, :])
    w1b_sb = const.tile([128, D], FT)
    nc.scalar.dma_start(out=w1b_sb, in_=w1b[:, :])
    w2_sb = const.tile([128, 1], FT)
    nc.sync.dma_start(out=w2_sb, in_=w2[:, :])
    b1_sb = const.tile([128, 1], FT)
    nc.gpsimd.dma_start(out=b1_sb, in_=b1[:, :])
    b2h_sb = const.tile([128, 1], FT)
    nc.sync.dma_start(out=b2h_sb, in_=b2h[:, :])
    v_sb = sb_in.tile([128, JCH, D], FT)
    vr = v[:, :].rearrange("(c p) d -> p c d", c=JCH)
    for jc in range(JCH):
        dmae[jc % 3].dma_start(out=v_sb[:, jc, :], in_=vr[:, jc, :])
    if am is not None:
        m_sb = sb_in.tile([128, NCHUNK, L], FT)
        mr = am[:, :].rearrange("(c p) j -> p c j", c=NCHUNK)
        for icn in range(NCHUNK):
            dmae[icn % 2].dma_start(out=m_sb[:, icn, :], in_=mr[:, icn, :])

    # ---- constants built on-chip ----
    ident = const.tile([128, 128], FT)
    make_identity(nc, ident)

    # w2pad[k, r, m] = w2[k] * (r == m): block-diagonal lhsT slices used to
    # scatter per-query conflict rows into 32-row PSUM col-groups.
    w2h_sb = const.tile([128, 1], BF)
    nc.vector.tensor_copy(w2h_sb, w2_sb)
    w2pad = const.tile([128, 32, 32], BF)
    nc.vector.memset(w2pad, 0.0)
    w2flat = w2pad.rearrange("p a b -> p (a b)")
    diag = bass.AP(tensor=w2flat.tensor, offset=w2flat.offset,
                   ap=[list(w2flat.ap[0]), [33, 32]])
    w2bc = bass.AP(tensor=w2h_sb.tensor, offset=w2h_sb.offset,
                   ap=[list(w2h_sb.ap[0]), [0, 32]])
    nc.vector.tensor_copy(diag, w2bc)

    w1a16 = const.tile([128, D], BF)
    nc.vector.tensor_copy(w1a16, w1a_sb)
    w1b16 = const.tile([128, D], BF)
    nc.vector.tensor_copy(w1b16, w1b_sb)

    # ---- transposes: fp16 copies (ACT) for the h-path, fp32 (DVE) for scores
    kT = sb_mid.tile([128, L], FT)
    kT_h = sb_mid.tile([128, L], BF)
    for jc in range(JCH):
        pst = ps_sm.tile([128, 128], FT, tag="tr")
        nc.tensor.transpose(pst, k_sb[:, jc, :], ident)
        nc.scalar.copy(kT_h[:, jc * 128 : (jc + 1) * 128], pst)
        nc.vector.tensor_copy(kT[:, jc * 128 : (jc + 1) * 128], pst)
    qT = sb_mid.tile([128, IPC], FT)
    qT_h = sb_mid.tile([128, IPC], BF)
    for icn in range(NCHUNK):
        pst = ps_sm.tile([128, 128], FT, tag="tr")
        nc.tensor.transpose(pst, q_sb[:, icn, :], ident)
        nc.scalar.copy(qT_h[:, icn * 128 : (icn + 1) * 128], pst)
        nc.vector.tensor_copy(qT[:, icn * 128 : (icn + 1) * 128], pst)

    # ---- hqT = W1a^T Q^T (fp32 out); hkbT = W1b^T K^T + b1 (fp16) ----
    psh = ps_big.tile([128, L], FT, tag="big")
    nc.tensor.matmul(psh[:, :IPC], lhsT=w1a16, rhs=qT_h, start=True, stop=True)
    hqT = sb_mid.tile([128, IPC], FT)
    nc.scalar.copy(hqT, psh[:, :IPC])

    psk = ps_big.tile([128, L], FT, tag="big")
    nc.tensor.matmul(psk, lhsT=w1b16, rhs=kT_h, start=True, stop=True)
    hkbT = sb_mid.tile([128, L], BF)
    nc.scalar.add(hkbT, psk, b1_sb)

    sscale = 0.5 / math.sqrt(D)

    # ---- main loop over query chunks ----
    for ic in range(NCHUNK):
        cps = ps_big.tile([128, L], FT, tag="big")
        s_sb = sb_mid.tile([128, L], FT, tag=f"scores{ic}")
        for r in range(32):
            for g in range(4):
                i = 32 * g + r
                gi = ic * 128 + i
                rt = relu_p.tile([128, L], BF, tag="rt")
                if g == 3:
                    nc.scalar.activation(rt, hkbT, AF.Relu, bias=hqT[:, gi : gi + 1])
                elif g == 2:
                    nc.gpsimd.tensor_scalar(
                        out=rt, in0=hkbT, scalar1=hqT[:, gi : gi + 1],
                        scalar2=0.0, op0=ALU.add, op1=ALU.max,
                    )
                else:
                    nc.vector.tensor_scalar(
                        out=rt, in0=hkbT, scalar1=hqT[:, gi : gi + 1],
                        scalar2=0.0, op0=ALU.add, op1=ALU.max,
                    )
                nc.tensor.matmul(
                    cps[32 * g : 32 * (g + 1), :],
                    lhsT=w2pad[:, r, :],
                    rhs=rt,
                    start=(r == 0),
                    stop=(r == 31),
                    tile_position=(0, 32 * g),
                    skip_group_check=True,
                )
            if r == 0:
                # deferred off the startup critical path: scores for this chunk
                pss = ps_big.tile([128, L], FT, tag="big")
                nc.tensor.matmul(
                    pss, lhsT=qT[:, ic * 128 : (ic + 1) * 128], rhs=kT,
                    start=True, stop=True,
                )
                nc.scalar.mul(s_sb, pss, sscale)

        # gate: t = tanh(0.5*conflict + b2/2); gated = (t + 1) * s  (s pre-scaled)
        t_sb = tmp_p.tile([128, L], FT, tag="t")
        nc.scalar.activation(t_sb, cps, AF.Tanh, bias=b2h_sb, scale=0.5)
        g_sb = tmp_p.tile([128, L], FT, tag="g")
        nc.vector.scalar_tensor_tensor(
            out=g_sb, in0=t_sb, scalar=1.0, in1=s_sb, op0=ALU.add, op1=ALU.mult
        )
        if am is not None:
            nc.vector.tensor_add(g_sb, g_sb, m_sb[:, ic, :])

        # softmax over j (no max-subtraction: |gated| <= ~8 so exp is safe)
        e_sb = tmp_p.tile([128, L], FT, tag="e")
        ssum = small_p.tile([128, 1], FT, tag="ssum")
        nc.scalar.activation(e_sb, g_sb, AF.Exp, accum_out=ssum)
        rec = small_p.tile([128, 1], FT, tag="rec")
        nc.vector.reciprocal(rec, ssum)
        a_sb = attn_p.tile([128, L], FT, tag="attn")
        nc.vector.tensor_scalar_mul(a_sb, e_sb, rec)
        for jc in range(JCH):
            dmae[jc % 3].dma_start(
                out=attn[ic * 128 : (ic + 1) * 128, jc * 128 : (jc + 1) * 128],
                in_=a_sb[:, jc * 128 : (jc + 1) * 128],
            )

        # out = attn @ V via PE transposes of attn
        po = ps_sm.tile([128, 128], FT, tag="out")
        for jc in range(JCH):
            pst = ps_sm.tile([128, 128], FT, tag="tr")
            nc.tensor.transpose(pst, a_sb[:, jc * 128 : (jc + 1) * 128], ident)
            at_sb = att_t.tile([128, 128], FT, tag="at")
            if jc % 2 == 0:
                nc.vector.tensor_copy(at_sb, pst)
            else:
                nc.scalar.copy(at_sb, pst)
            nc.tensor.matmul(
                po, lhsT=at_sb, rhs=v_sb[:, jc, :], start=(jc == 0), stop=(jc == JCH - 1)
            )
        o_sb = att_t.tile([128, D], FT, tag="o")
        nc.scalar.copy(o_sb, po)
        nc.sync.dma_start(out=out[ic * 128 : (ic + 1) * 128, :], in_=o_sb)


def _get_module(masked: bool) -> bass.Bass:
    if masked not in _module_cache:
        _module_cache[masked] = _build_module(masked)
    return _module_cache[masked]


def kernel(**inputs) -> tuple:
    Q = np.ascontiguousarray(np.asarray(inputs["Q"], dtype=np.float32))
    K = np.ascontiguousarray(np.asarray(inputs["K"], dtype=np.float32))
    V = np.ascontiguousarray(np.asarray(inputs["V"], dtype=np.float32))
    mask = np.asarray(inputs["attention_mask"])
    W1 = np.asarray(inputs["W1"], dtype=np.float32)
    b1 = np.asarray(inputs["b1"], dtype=np.float32)
    W2 = np.asarray(inputs["W2"], dtype=np.float32)
    b2 = np.asarray(inputs["b2"], dtype=np.float32)

    masked = bool((mask == 0).any())
    nc = _get_module(masked)

    w1a = np.ascontiguousarray(W1[:D, :])
    w1b = np.ascontiguousarray(W1[D:, :])
    w2c = np.ascontiguousarray(W2.reshape(D, 1))
    b1c = np.ascontiguousarray(b1.reshape(D, 1))
    b2h = np.full((D, 1), float(b2.reshape(-1)[0]) / 2.0, dtype=np.float32)
    if masked:
        addmask = np.where(mask == 0, np.float32(-1e30), np.float32(0.0)).astype(
            np.float32
        )

    in_maps = []
    for c in range(NCORES):
        b, i0 = c // 2, (c % 2) * IPC
        im = {
            "q": np.ascontiguousarray(Q[b, i0 : i0 + IPC, :]),
            "k": K[b],
            "v": V[b],
            "w1a": w1a,
            "w1b": w1b,
            "w2": w2c,
            "b1": b1c,
            "b2h": b2h,
        }
        if masked:
            im["am"] = np.ascontiguousarray(addmask[b, i0 : i0 + IPC, :])
        in_maps.append(im)

    res = run_bass_kernel_spmd(nc, in_maps, list(range(NCORES)))
    global _last_results
    _last_results = res

    out = np.empty((B, L, D), dtype=np.float32)
    attn = np.empty((B, L, L), dtype=np.float32)
    for c in range(NCORES):
        b, i0 = c // 2, (c % 2) * IPC
        out[b, i0 : i0 + IPC, :] = res.results[c]["out"]
        attn[b, i0 : i0 + IPC, :] = res.results[c]["attn"]
    return out, attn


# revision 18
# speedup vs baseline: 6.0490x; 6.0490x over previous
"""ContextGateAttention TRN2 kernel.

B=4, L=512, D=128. 8 NeuronCores, data-parallel: core c owns batch c//2,
query rows (c%2)*256 .. +256. No collectives.

Per-core pipeline (all in [partition, free] layouts):
  KT/QT       : PE transposes of K,Q tiles              -> [d, j], [d, i]
  hqT         : W1a^T-matmul                            -> [d2, i]  (256)
  hkbT        : W1b-matmul + b1 bias                    -> [d2, j]  (512)
  scores      : QT^T @ KT, scaled by 0.5/sqrt(D)        -> [i, j]
  conflict    : per query i: relu(hkbT + hqT[:,i]) on DVE/ACT (fused
                tensor_scalar add+max / activation Relu+bias), then a
                M=1 PE matmul against w2 reduces over d -> PSUM row i
  gate        : sigmoid(c + b2) = 0.5*(1 + tanh(c/2 + b2/2)); with the
                0.5 folded into the score scale: gated = s*t + s
  softmax     : reduce_max -> exp(bias=-max, accum_out=sum) -> mul 1/sum
  out         : PE transpose of attn -> attn^T @ V (4-step PSUM accum)
"""

import math
from contextlib import ExitStack

import numpy as np

import concourse.bass as bass
import concourse.bacc as bacc
import concourse.mybir as mybir
import concourse.tile as tile
from concourse.bass_utils import run_bass_kernel_spmd
from concourse.masks import make_identity

B, L, D = 4, 512, 128
NCORES = 8
IPC = L * B // NCORES  # 256 query rows per core
NCHUNK = IPC // 128  # 2 partition chunks of query rows
FT = mybir.dt.float32
BF = mybir.dt.float16
AF = mybir.ActivationFunctionType
ALU = mybir.AluOpType

_module_cache: dict = {}


def _build_module(masked: bool) -> bass.Bass:
    nc = bacc.Bacc()
    q = nc.declare_dram_parameter("q", [IPC, D], FT, isOutput=False)
    k = nc.declare_dram_parameter("k", [L, D], FT, isOutput=False)
    v = nc.declare_dram_parameter("v", [L, D], FT, isOutput=False)
    w1a = nc.declare_dram_parameter("w1a", [D, D], FT, isOutput=False)
    w1b = nc.declare_dram_parameter("w1b", [D, D], FT, isOutput=False)
    w2 = nc.declare_dram_parameter("w2", [D, 1], FT, isOutput=False)
    b1 = nc.declare_dram_parameter("b1", [D, 1], FT, isOutput=False)
    b2h = nc.declare_dram_parameter("b2h", [D, 1], FT, isOutput=False)
    am = (
        nc.declare_dram_parameter("am", [IPC, L], FT, isOutput=False)
        if masked
        else None
    )
    attn = nc.declare_dram_parameter("attn", [IPC, L], FT, isOutput=True)
    out = nc.declare_dram_parameter("out", [IPC, D], FT, isOutput=True)

    with tile.TileContext(nc) as tc:
        with ExitStack() as ctx:
            _body(ctx, tc, q, k, v, w1a, w1b, w2, b1, b2h, am, attn, out)
    nc.compile()
    return nc


def _body(ctx, tc, q, k, v, w1a, w1b, w2, b1, b2h, am, attn, out):
    nc = tc.nc
    JCH = L // 128  # 4 key chunks
    dmae = [nc.sync, nc.gpsimd, nc.scalar]

    const = ctx.enter_context(tc.tile_pool(name="const", bufs=1))
    sb_in = ctx.enter_context(tc.tile_pool(name="sb_in", bufs=1))
    sb_mid = ctx.enter_context(tc.tile_pool(name="sb_mid", bufs=1))
    relu_p = ctx.enter_context(tc.tile_pool(name="relu", bufs=10))
    tmp_p = ctx.enter_context(tc.tile_pool(name="tmp", bufs=4))
    small_p = ctx.enter_context(tc.tile_pool(name="small", bufs=8))
    attn_p = ctx.enter_context(tc.tile_pool(name="attn", bufs=2))
    att_t = ctx.enter_context(tc.tile_pool(name="att_t", bufs=8))
    ps_big = ctx.enter_context(tc.tile_pool(name="ps_big", bufs=4, space="PSUM"))
    ps_sm = ctx.enter_context(tc.tile_pool(name="ps_sm", bufs=2, space="PSUM"))

    # ---- input DMAs first, spread across engine queues ----
    k_sb = sb_in.tile([128, JCH, D], FT)
    kr = k[:, :].rearrange("(c p) d -> p c d", c=JCH)
    for jc in range(JCH):
        dmae[jc % 3].dma_start(out=k_sb[:, jc, :], in_=kr[:, jc, :])
    q_sb = sb_in.tile([128, NCHUNK, D], FT)
    qr = q[:, :].rearrange("(c p) d -> p c d", c=NCHUNK)
    for icn in range(NCHUNK):
        dmae[icn % 3].dma_start(out=q_sb[:, icn, :], in_=qr[:, icn, :])
    w1a_sb = const.tile([128, D], FT)
    nc.sync.dma_start(out=w1a_sb, in_=w1a[:, :])
    w1b_sb = const.tile([128, D], FT)
    nc.scalar.dma_start(out=w1b_sb, in_=w1b[:, :])
    w2_sb = const.tile([128, 1], FT)
    nc.sync.dma_start(out=w2_sb, in_=w2[:, :])
    b1_sb = const.tile([128, 1], FT)
    nc.gpsimd.dma_start(out=b1_sb, in_=b1[:, :])
    b2h_sb = const.tile([128, 1], FT)
    nc.sync.dma_start(out=b2h_sb, in_=b2h[:, :])
    v_sb = sb_in.tile([128, JCH, D], FT)
    vr = v[:, :].rearrange("(c p) d -> p c d", c=JCH)
    for jc in range(JCH):
        dmae[jc % 3].dma_start(out=v_sb[:, jc, :], in_=vr[:, jc, :])
    if am is not None:
        m_sb = sb_in.tile([128, NCHUNK, L], FT)
        mr = am[:, :].rearrange("(c p) j -> p c j", c=NCHUNK)
        for icn in range(NCHUNK):
            dmae[icn % 2].dma_start(out=m_sb[:, icn, :], in_=mr[:, icn, :])

    # ---- constants built on-chip ----
    ident = const.tile([128, 128], FT)
    make_identity(nc, ident)

    # w2pad[k, r, m] = w2[k] * (r == m): block-diagonal lhsT slices used to
    # scatter per-query conflict rows into 32-row PSUM col-groups.
    w2h_sb = const.tile([128, 1], BF)
    nc.vector.tensor_copy(w2h_sb, w2_sb)
    w2pad = const.tile([128, 32, 32], BF)
    nc.vector.memset(w2pad, 0.0)
    w2flat = w2pad.rearrange("p a b -> p (a b)")
    diag = bass.AP(tensor=w2flat.tensor, offset=w2flat.offset,
                   ap=[list(w2flat.ap[0]), [33, 32]])
    w2bc = bass.AP(tensor=w2h_sb.tensor, offset=w2h_sb.offset,
                   ap=[list(w2h_sb.ap[0]), [0, 32]])
    nc.vector.tensor_copy(diag, w2bc)

    w1a16 = const.tile([128, D], BF)
    nc.vector.tensor_copy(w1a16, w1a_sb)
    w1b16 = const.tile([128, D], BF)
    nc.vector.tensor_copy(w1b16, w1b_sb)

    # ---- transposes: fp16 copies (ACT) for the h-path, fp32 (DVE) for scores
    kT = sb_mid.tile([128, L], FT)
    kT_h = sb_mid.tile([128, L], BF)
    for jc in range(JCH):
        pst = ps_sm.tile([128, 128], FT, tag="tr")
        nc.tensor.transpose(pst, k_sb[:, jc, :], ident)
        nc.scalar.copy(kT_h[:, jc * 128 : (jc + 1) * 128], pst)
        nc.vector.tensor_copy(kT[:, jc * 128 : (jc + 1) * 128], pst)
    qT = sb_mid.tile([128, IPC], FT)
    qT_h = sb_mid.tile([128, IPC], BF)
    for icn in range(NCHUNK):
        pst = ps_sm.tile([128, 128], FT, tag="tr")
        nc.tensor.transpose(pst, q_sb[:, icn, :], ident)
        nc.scalar.copy(qT_h[:, icn * 128 : (icn + 1) * 128], pst)
        nc.vector.tensor_copy(qT[:, icn * 128 : (icn + 1) * 128], pst)

    # ---- hqT = W1a^T Q^T (fp32 out); hkbT = W1b^T K^T + b1 (fp16) ----
    psh = ps_big.tile([128, L], FT, tag="big")
    nc.tensor.matmul(psh[:, :IPC], lhsT=w1a16, rhs=qT_h, start=True, stop=True)
    hqT = sb_mid.tile([128, IPC], FT)
    nc.scalar.copy(hqT, psh[:, :IPC])

    psk = ps_big.tile([128, L], FT, tag="big")
    nc.tensor.matmul(psk, lhsT=w1b16, rhs=kT_h, start=True, stop=True)
    hkbT = sb_mid.tile([128, L], BF)
    nc.scalar.add(hkbT, psk, b1_sb)

    sscale = 0.5 / math.sqrt(D)

    # ---- main loop over query chunks ----
    for ic in range(NCHUNK):
        cps = ps_big.tile([128, L], FT, tag="big")
        s_sb = sb_mid.tile([128, L], FT, tag=f"scores{ic}")
        for r in range(32):
            for g in range(4):
                i = 32 * g + r
                gi = ic * 128 + i
                rt = relu_p.tile([128, L], BF, tag="rt")
                if g == 3:
                    nc.scalar.activation(rt, hkbT, AF.Relu, bias=hqT[:, gi : gi + 1])
                else:
                    nc.vector.tensor_scalar(
                        out=rt, in0=hkbT, scalar1=hqT[:, gi : gi + 1],
                        scalar2=0.0, op0=ALU.add, op1=ALU.max,
                    )
                nc.tensor.matmul(
                    cps[32 * g : 32 * (g + 1), :],
                    lhsT=w2pad[:, r, :],
                    rhs=rt,
                    start=(r == 0),
                    stop=(r == 31),
                    tile_position=(0, 32 * g),
                    skip_group_check=True,
                )
            if r == 0:
                # deferred off the startup critical path: scores for this chunk
                pss = ps_big.tile([128, L], FT, tag="big")
                nc.tensor.matmul(
                    pss, lhsT=qT[:, ic * 128 : (ic + 1) * 128], rhs=kT,
                    start=True, stop=True,
                )
                nc.scalar.mul(s_sb, pss, sscale)

        # gate: t = tanh(0.5*conflict + b2/2); gated = (t + 1) * s  (s pre-scaled)
        t_sb = tmp_p.tile([128, L], FT, tag="t")
        nc.scalar.activation(t_sb, cps, AF.Tanh, bias=b2h_sb, scale=0.5)
        g_sb = tmp_p.tile([128, L], FT, tag="g")
        nc.vector.scalar_tensor_tensor(
            out=g_sb, in0=t_sb, scalar=1.0, in1=s_sb, op0=ALU.add, op1=ALU.mult
        )
        if am is not None:
            nc.vector.tensor_add(g_sb, g_sb, m_sb[:, ic, :])

        # softmax over j (no max-subtraction: |gated| <= ~8 so exp is safe)
        e_sb = tmp_p.tile([128, L], FT, tag="e")
        ssum = small_p.tile([128, 1], FT, tag="ssum")
        nc.scalar.activation(e_sb, g_sb, AF.Exp, accum_out=ssum)
        rec = small_p.tile([128, 1], FT, tag="rec")
        nc.vector.reciprocal(rec, ssum)
        a_sb = attn_p.tile([128, L], FT, tag="attn")
        nc.vector.tensor_scalar_mul(a_sb, e_sb, rec)
        for jc in range(JCH):
            dmae[jc % 3].dma_start(
                out=attn[ic * 128 : (ic + 1) * 128, jc * 128 : (jc + 1) * 128],
                in_=a_sb[:, jc * 128 : (jc + 1) * 128],
            )

        # out = attn @ V via PE transposes of attn
        po = ps_sm.tile([128, 128], FT, tag="out")
        for jc in range(JCH):
            pst = ps_sm.tile([128, 128], FT, tag="tr")
            nc.tensor.transpose(pst, a_sb[:, jc * 128 : (jc + 1) * 128], ident)
            at_sb = att_t.tile([128, 128], FT, tag="at")
            if jc % 2 == 0:
                nc.vector.tensor_copy(at_sb, pst)
            else:
                nc.scalar.copy(at_sb, pst)
            nc.tensor.matmul(
                po, lhsT=at_sb, rhs=v_sb[:, jc, :], start=(jc == 0), stop=(jc == JCH - 1)
            )
        o_sb = att_t.tile([128, D], FT, tag="o")
        nc.scalar.copy(o_sb, po)
        nc.sync.dma_start(out=out[ic * 128 : (ic + 1) * 128, :], in_=o_sb)


def _get_module(masked: bool) -> bass.Bass:
    if masked not in _module_cache:
        _module_cache[masked] = _build_module(masked)
    return _module_cache[masked]


def kernel(**inputs) -> tuple:
    Q = np.ascontiguousarray(np.asarray(inputs["Q"], dtype=np.float32))
    K = np.ascontiguousarray(np.asarray(inputs["K"], dtype=np.float32))
    V = np.ascontiguousarray(np.asarray(inputs["V"], dtype=np.float32))
    mask = np.asarray(inputs["attention_mask"])
    W1 = np.asarray(inputs["W1"], dtype=np.float32)
    b1 = np.asarray(inputs["b1"], dtype=np.float32)
    W2 = np.asarray(inputs["W2"], dtype=np.float32)
    b2 = np.asarray(inputs["b2"], dtype=np.float32)

    masked = bool((mask == 0).any())
    nc = _get_module(masked)

    w1a = np.ascontiguousarray(W1[:D, :])
    w1b = np.ascontiguousarray(W1[D:, :])
    w2c = np.ascontiguousarray(W2.reshape(D, 1))
    b1c = np.ascontiguousarray(b1.reshape(D, 1))
    b2h = np.full((D, 1), float(b2.reshape(-1)[0]) / 2.0, dtype=np.float32)
    if masked:
        addmask = np.where(mask == 0, np.float32(-1e30), np.float32(0.0)).astype(
            np.float32
        )

    in_maps = []
    for c in range(NCORES):
        b, i0 = c // 2, (c % 2) * IPC
        im = {
            "q": np.ascontiguousarray(Q[b, i0 : i0 + IPC, :]),
            "k": K[b],
            "v": V[b],
            "w1a": w1a,
            "w1b": w1b,
            "w2": w2c,
            "b1": b1c,
            "b2h": b2h,
        }
        if masked:
            im["am"] = np.ascontiguousarray(addmask[b, i0 : i0 + IPC, :])
        in_maps.append(im)

    res = run_bass_kernel_spmd(nc, in_maps, list(range(NCORES)))
    global _last_results
    _last_results = res

    out = np.empty((B, L, D), dtype=np.float32)
    attn = np.empty((B, L, L), dtype=np.float32)
    for c in range(NCORES):
        b, i0 = c // 2, (c % 2) * IPC
        out[b, i0 : i0 + IPC, :] = res.results[c]["out"]
        attn[b, i0 : i0 + IPC, :] = res.results[c]["attn"]
    return out, attn
